# revision 3
# baseline (speedup 1.0000x reference)
"""Trainium2 Bass kernel for nn_Attention_13297218748956.

Multi-head causal self-attention with RoPE (B=64, T=128, C=2048, H=16, hd=128),
fp32, data-parallel over batch across 8 NeuronCores (8 batches/core, no
collectives).

Per-core layout strategy (host preps transposed operands so the PE contraction
dim lands on partitions):
  xT  [C, 1024]   : x shard transposed (1024 = 8 batches x 128 tokens)
  wqT/wkT/wvT/woT : weight transposes [C, C]
  phase 1a: QT/KT head tiles = (wT col-block).T @ xT, RoPE applied in
            [d, t] layout via a pair-swap matmul (rot = ST.T @ q) plus
            cos/sin elementwise; 1/sqrt(hd) folded into Q's cos/sin.
  phase 1b: V = xT.T @ wvT in natural [t, d] layout.
  phase 2:  per (batch, head): scores = QT.T @ KT -> +mask -> softmax along
            free dim -> PE-transpose(probs) -> out_headT = V.T @ probsT.
  phase 3:  per batch: y = attnT.T @ woT accumulated over head chunks.
"""

import numpy as np

import concourse.bacc as bacc
import concourse.tile as tile
import concourse.mybir as mybir
from concourse.bass_utils import run_bass_kernel_spmd

N_CORES = 8
B, T, C, H = 64, 128, 2048, 16
HD = C // H  # 128
BPC = B // N_CORES  # batches per core
TOK = BPC * T  # tokens per core (1024)
KC = C // 128  # contraction chunks (16)
TH = TOK // 512  # token halves for 512-wide fp32 moving operand (2)
F32 = mybir.dt.float32

_CACHE = {}


def _build():
    nc = bacc.Bacc("TRN2", target_bir_lowering=False, debug=False)

    xT = nc.dram_tensor("xT", [C, TOK], F32, kind="ExternalInput")
    wqT = nc.dram_tensor("wqT", [C, C], F32, kind="ExternalInput")
    wkT = nc.dram_tensor("wkT", [C, C], F32, kind="ExternalInput")
    wvT = nc.dram_tensor("wvT", [C, C], F32, kind="ExternalInput")
    woT = nc.dram_tensor("woT", [C, C], F32, kind="ExternalInput")
    stmat = nc.dram_tensor("stmat", [128, 128], F32, kind="ExternalInput")
    ident = nc.dram_tensor("ident", [128, 128], F32, kind="ExternalInput")
    maskd = nc.dram_tensor("maskd", [128, 128], F32, kind="ExternalInput")
    cosq = nc.dram_tensor("cosq", [128, 512], F32, kind="ExternalInput")
    sinq = nc.dram_tensor("sinq", [128, 512], F32, kind="ExternalInput")
    cosk = nc.dram_tensor("cosk", [128, 512], F32, kind="ExternalInput")
    sink = nc.dram_tensor("sink", [128, 512], F32, kind="ExternalInput")
    y = nc.dram_tensor("y", [TOK, C], F32, kind="ExternalOutput")

    with tile.TileContext(nc) as tc:
        with (
            tc.tile_pool(name="consts", bufs=1) as consts,
            tc.tile_pool(name="dram", bufs=1, space="DRAM") as dram,
        ):
            st_t = consts.tile([128, 128], F32)
            id_t = consts.tile([128, 128], F32)
            mask_t = consts.tile([128, 128], F32)
            cosq_t = consts.tile([128, 512], F32)
            sinq_t = consts.tile([128, 512], F32)
            cosk_t = consts.tile([128, 512], F32)
            sink_t = consts.tile([128, 512], F32)
            for t_, d_ in [
                (st_t, stmat), (id_t, ident), (mask_t, maskd),
                (cosq_t, cosq), (sinq_t, sinq), (cosk_t, cosk), (sink_t, sink),
            ]:
                nc.sync.dma_start(out=t_[:], in_=d_[:])

            qs = dram.tile([C, TOK], F32)  # roped QT
            ks = dram.tile([C, TOK], F32)  # roped KT
            vs = dram.tile([TOK, C], F32)  # V natural layout

            # ---------------- phase 1: projections ----------------
            with tc.tile_pool(name="xt", bufs=1) as xtp:
                xt = []
                for k in range(KC):
                    xk = xtp.tile([128, TOK], F32, tag=f"xt{k}")
                    nc.sync.dma_start(out=xk[:], in_=xT[k * 128:(k + 1) * 128, :])
                    xt.append(xk)

                # --- 1a: QT / KT with RoPE ---
                with (
                    tc.tile_pool(name="wcol", bufs=2) as wcolp,
                    tc.tile_pool(name="p1stage", bufs=4) as stagep,
                    tc.tile_pool(name="ropetmp", bufs=4) as ropep,
                    tc.tile_pool(name="psqk", bufs=3, space="PSUM") as psqk,
                    tc.tile_pool(name="psrot", bufs=2, space="PSUM") as psrot,
                ):
                    for wT, cos_t, sin_t, outd in (
                        (wqT, cosq_t, sinq_t, qs),
                        (wkT, cosk_t, sink_t, ks),
                    ):
                        for n in range(H):
                            wcol = wcolp.tile([128, KC, 128], F32, tag="wcol")
                            nc.sync.dma_start(
                                out=wcol[:],
                                in_=wT[:, n * 128:(n + 1) * 128].rearrange(
                                    "(kc p) n -> p kc n", p=128
                                ),
                            )
                            for th in range(TH):
                                acc = psqk.tile([128, 512], F32, tag="psqk")
                                for k in range(KC):
                                    nc.tensor.matmul(
                                        acc[:],
                                        wcol[:, k, :],
                                        xt[k][:, th * 512:(th + 1) * 512],
                                        start=(k == 0),
                                        stop=(k == KC - 1),
                                    )
                                qt_sb = stagep.tile([128, 512], F32, tag="qt_sb")
                                nc.scalar.copy(out=qt_sb[:], in_=acc[:])
                                rot = psrot.tile([128, 512], F32, tag="rot")
                                nc.tensor.matmul(
                                    rot[:], st_t[:], qt_sb[:], start=True, stop=True
                                )
                                t1 = ropep.tile([128, 512], F32, tag="t1")
                                nc.vector.tensor_mul(t1[:], qt_sb[:], cos_t[:])
                                t2 = ropep.tile([128, 512], F32, tag="t2")
                                nc.vector.tensor_mul(t2[:], rot[:], sin_t[:])
                                qr = stagep.tile([128, 512], F32, tag="qr")
                                nc.vector.tensor_add(qr[:], t1[:], t2[:])
                                nc.sync.dma_start(
                                    out=outd[n * 128:(n + 1) * 128,
                                             th * 512:(th + 1) * 512],
                                    in_=qr[:],
                                )

                # --- 1b: V ---
                with (
                    tc.tile_pool(name="wvset", bufs=2) as wvsp,
                    tc.tile_pool(name="vstage", bufs=4) as vstagep,
                    tc.tile_pool(name="psv", bufs=4, space="PSUM") as psv,
                ):
                    for m in range(4):
                        wvs = wvsp.tile([128, KC, 512], F32, tag="wvs")
                        nc.sync.dma_start(
                            out=wvs[:],
                            in_=wvT[:, m * 512:(m + 1) * 512].rearrange(
                                "(kc p) n -> p kc n", p=128
                            ),
                        )
                        for tt in range(BPC):
                            acc = psv.tile([128, 512], F32, tag="psv")
                            for k in range(KC):
                                nc.tensor.matmul(
                                    acc[:],
                                    xt[k][:, tt * 128:(tt + 1) * 128],
                                    wvs[:, k, :],
                                    start=(k == 0),
                                    stop=(k == KC - 1),
                                )
                            v_sb = vstagep.tile([128, 512], F32, tag="v_sb")
                            nc.scalar.copy(out=v_sb[:], in_=acc[:])
                            nc.sync.dma_start(
                                out=vs[tt * 128:(tt + 1) * 128,
                                       m * 512:(m + 1) * 512],
                                in_=v_sb[:],
                            )

            # ---------------- phase 2+3: attention + output proj ----------------
            with (
                tc.tile_pool(name="wo", bufs=1) as wop,
                tc.tile_pool(name="qkv", bufs=4) as qkvp,
                tc.tile_pool(name="soft", bufs=3) as softp,
                tc.tile_pool(name="small", bufs=4) as smallp,
                tc.tile_pool(name="attnt", bufs=2) as attntp,
                tc.tile_pool(name="ystage", bufs=3) as ystagep,
                tc.tile_pool(name="psatt", bufs=2, space="PSUM") as psatt,
                tc.tile_pool(name="psy", bufs=2, space="PSUM") as psy,
            ):
                wo_t = wop.tile([128, KC, C], F32)
                nc.sync.dma_start(
                    out=wo_t[:],
                    in_=woT.rearrange("(kc p) n -> p kc n", p=128),
                )
                for b in range(BPC):
                    attnT = attntp.tile([128, H, 128], F32, tag="attnT")
                    for h in range(H):
                        q_t = qkvp.tile([128, 128], F32, tag="q")
                        nc.sync.dma_start(
                            out=q_t[:],
                            in_=qs[h * 128:(h + 1) * 128, b * 128:(b + 1) * 128],
                        )
                        k_t = qkvp.tile([128, 128], F32, tag="k")
                        nc.sync.dma_start(
                            out=k_t[:],
                            in_=ks[h * 128:(h + 1) * 128, b * 128:(b + 1) * 128],
                        )
                        v_t = qkvp.tile([128, 128], F32, tag="v")
                        nc.sync.dma_start(
                            out=v_t[:],
                            in_=vs[b * 128:(b + 1) * 128, h * 128:(h + 1) * 128],
                        )
                        sc = psatt.tile([128, 128], F32, tag="sc")
                        nc.tensor.matmul(sc[:], q_t[:], k_t[:], start=True, stop=True)
                        masked = softp.tile([128, 128], F32, tag="masked")
                        nc.vector.tensor_add(masked[:], sc[:], mask_t[:])
                        negmax = smallp.tile([128, 1], F32, tag="negmax")
                        nc.vector.reduce_max(
                            out=negmax[:], in_=masked[:],
                            axis=mybir.AxisListType.X, negate=True,
                        )
                        e_t = softp.tile([128, 128], F32, tag="e")
                        sums = smallp.tile([128, 1], F32, tag="sums")
                        nc.scalar.activation(
                            out=e_t[:], in_=masked[:],
                            func=mybir.ActivationFunctionType.Exp,
                            bias=negmax[:], scale=1.0, accum_out=sums[:],
                        )
                        inv = smallp.tile([128, 1], F32, tag="inv")
                        nc.vector.reciprocal(out=inv[:], in_=sums[:])
                        probs = softp.tile([128, 128], F32, tag="probs")
                        nc.vector.tensor_scalar_mul(probs[:], e_t[:], inv[:])
                        pT = psatt.tile([128, 128], F32, tag="pT")
                        nc.tensor.transpose(pT[:], probs[:], id_t[:])
                        pT_sb = softp.tile([128, 128], F32, tag="pT_sb")
                        nc.scalar.copy(out=pT_sb[:], in_=pT[:])
                        pv = psatt.tile([128, 128], F32, tag="pv")
                        nc.tensor.matmul(pv[:], v_t[:], pT_sb[:], start=True, stop=True)
                        nc.scalar.copy(out=attnT[:, h, :], in_=pv[:])
                    for m in range(4):
                        yacc = psy.tile([128, 512], F32, tag="yacc")
                        for h in range(H):
                            nc.tensor.matmul(
                                yacc[:],
                                attnT[:, h, :],
                                wo_t[:, h, m * 512:(m + 1) * 512],
                                start=(h == 0),
                                stop=(h == H - 1),
                            )
                        y_sb = ystagep.tile([128, 512], F32, tag="y_sb")
                        nc.vector.tensor_copy(y_sb[:], yacc[:])
                        nc.sync.dma_start(
                            out=y[b * 128:(b + 1) * 128, m * 512:(m + 1) * 512],
                            in_=y_sb[:],
                        )

    nc.compile()
    return nc


def _prep_inputs(x, freqs_cos, freqs_sin, wq, wk, wv, wo):
    x = np.asarray(x, dtype=np.float32)
    fc = np.asarray(freqs_cos, dtype=np.float32)
    fs = np.asarray(freqs_sin, dtype=np.float32)

    shared = {
        "wqT": np.ascontiguousarray(np.asarray(wq, np.float32).T),
        "wkT": np.ascontiguousarray(np.asarray(wk, np.float32).T),
        "wvT": np.ascontiguousarray(np.asarray(wv, np.float32).T),
        "woT": np.ascontiguousarray(np.asarray(wo, np.float32).T),
    }
    st = np.zeros((128, 128), np.float32)
    for j in range(64):
        st[2 * j + 1, 2 * j] = -1.0
        st[2 * j, 2 * j + 1] = 1.0
    shared["stmat"] = st
    shared["ident"] = np.eye(128, dtype=np.float32)
    shared["maskd"] = np.triu(np.full((128, 128), -1e30, np.float32), k=1)

    cosd = np.repeat(fc.T, 2, axis=0)  # [128, 128]: row d -> cos[t, d//2]
    sind = np.repeat(fs.T, 2, axis=0)
    cos4 = np.ascontiguousarray(np.tile(cosd, (1, 4)))  # [128, 512]
    sin4 = np.ascontiguousarray(np.tile(sind, (1, 4)))
    scale = np.float32(1.0 / np.sqrt(HD))
    shared["cosq"] = cos4 * scale
    shared["sinq"] = sin4 * scale
    shared["cosk"] = cos4
    shared["sink"] = sin4

    in_maps = []
    for i in range(N_CORES):
        shard = x[i * BPC:(i + 1) * BPC].reshape(TOK, C)
        m = dict(shared)
        m["xT"] = np.ascontiguousarray(shard.T)
        in_maps.append(m)
    return in_maps


def _run(inputs, trace=False):
    if "nc" not in _CACHE:
        _CACHE["nc"] = _build()
    nc = _CACHE["nc"]
    in_maps = _prep_inputs(**inputs)
    res = run_bass_kernel_spmd(
        nc, in_maps, core_ids=list(range(N_CORES)), trace=trace
    )
    out = np.empty((B, T, C), np.float32)
    for i in range(N_CORES):
        out[i * BPC:(i + 1) * BPC] = np.asarray(res.results[i]["y"]).reshape(
            BPC, T, C
        )
    return out, res


def kernel(**inputs):
    out, _ = _run(inputs, trace=False)
    return out


# revision 4
# speedup vs baseline: 2.5481x; 2.5481x over previous
"""Trainium2 Bass kernel for nn_Attention_13297218748956.

Multi-head causal self-attention with RoPE (B=64, T=128, C=2048, H=16, hd=128),
fp32, data-parallel over batch across 8 NeuronCores (8 batches/core, no
collectives).

Per-core layout strategy (host preps transposed operands so the PE contraction
dim lands on partitions):
  xT  [C, 1024]   : x shard transposed (1024 = 8 batches x 128 tokens)
  wqT/wkT/wvT/woT : weight transposes [C, C]
  phase 1a: QT/KT head tiles = (wT col-block).T @ xT, RoPE applied in
            [d, t] layout via a pair-swap matmul (rot = ST.T @ q) plus
            cos/sin elementwise; 1/sqrt(hd) folded into Q's cos/sin.
  phase 1b: V = xT.T @ wvT in natural [t, d] layout.
  phase 2:  per (batch, head): scores = QT.T @ KT -> +mask -> softmax along
            free dim -> PE-transpose(probs) -> out_headT = V.T @ probsT.
  phase 3:  per batch: y = attnT.T @ woT accumulated over head chunks.
"""

import numpy as np

import concourse.bacc as bacc
import concourse.tile as tile
import concourse.mybir as mybir
from concourse.bass_utils import run_bass_kernel_spmd

N_CORES = 8
B, T, C, H = 64, 128, 2048, 16
HD = C // H  # 128
BPC = B // N_CORES  # batches per core
TOK = BPC * T  # tokens per core (1024)
KC = C // 128  # contraction chunks (16)
TH = TOK // 512  # token halves for 512-wide fp32 moving operand (2)
F32 = mybir.dt.float32
F32R = mybir.dt.float32r
PROJ_DT = F32R  # f32r: 1 cyc/row PE matmul at N>=512 (vs 4 for fp32); measured ~2e-4 rel

_CACHE = {}


def _build():
    nc = bacc.Bacc("TRN2", target_bir_lowering=False, debug=False)

    xT = nc.dram_tensor("xT", [C, TOK], PROJ_DT, kind="ExternalInput")
    wqT = nc.dram_tensor("wqT", [C, C], PROJ_DT, kind="ExternalInput")
    wkT = nc.dram_tensor("wkT", [C, C], PROJ_DT, kind="ExternalInput")
    wvT = nc.dram_tensor("wvT", [C, C], PROJ_DT, kind="ExternalInput")
    woT = nc.dram_tensor("woT", [C, C], PROJ_DT, kind="ExternalInput")
    stmat = nc.dram_tensor("stmat", [128, 128], PROJ_DT, kind="ExternalInput")
    ident = nc.dram_tensor("ident", [128, 128], F32, kind="ExternalInput")
    maskd = nc.dram_tensor("maskd", [128, 128], F32, kind="ExternalInput")
    cosq = nc.dram_tensor("cosq", [128, 512], F32, kind="ExternalInput")
    sinq = nc.dram_tensor("sinq", [128, 512], F32, kind="ExternalInput")
    cosk = nc.dram_tensor("cosk", [128, 512], F32, kind="ExternalInput")
    sink = nc.dram_tensor("sink", [128, 512], F32, kind="ExternalInput")
    y = nc.dram_tensor("y", [TOK, C], F32, kind="ExternalOutput")

    with tile.TileContext(nc) as tc:
        with (
            tc.tile_pool(name="consts", bufs=1) as consts,
            tc.tile_pool(name="dram", bufs=1, space="DRAM") as dram,
        ):
            st_t = consts.tile([128, 128], PROJ_DT)
            id_t = consts.tile([128, 128], F32)
            mask_t = consts.tile([128, 128], F32)
            cosq_t = consts.tile([128, 512], F32)
            sinq_t = consts.tile([128, 512], F32)
            cosk_t = consts.tile([128, 512], F32)
            sink_t = consts.tile([128, 512], F32)
            for t_, d_ in [
                (st_t, stmat), (id_t, ident), (mask_t, maskd),
                (cosq_t, cosq), (sinq_t, sinq), (cosk_t, cosk), (sink_t, sink),
            ]:
                nc.sync.dma_start(out=t_[:], in_=d_[:])

            qs = dram.tile([C, TOK], F32)  # roped QT
            ks = dram.tile([C, TOK], F32)  # roped KT
            vs = dram.tile([TOK, C], F32)  # V natural layout

            # ---------------- phase 1: projections ----------------
            with tc.tile_pool(name="xt", bufs=1) as xtp:
                xt = []
                for k in range(KC):
                    xk = xtp.tile([128, TOK], PROJ_DT, tag=f"xt{k}")
                    nc.sync.dma_start(out=xk[:], in_=xT[k * 128:(k + 1) * 128, :])
                    xt.append(xk)

                # --- 1a: QT / KT with RoPE ---
                with (
                    tc.tile_pool(name="wcol", bufs=2) as wcolp,
                    tc.tile_pool(name="p1stage", bufs=4) as stagep,
                    tc.tile_pool(name="ropetmp", bufs=4) as ropep,
                    tc.tile_pool(name="psqk", bufs=3, space="PSUM") as psqk,
                    tc.tile_pool(name="psrot", bufs=2, space="PSUM") as psrot,
                ):
                    for wT, cos_t, sin_t, outd in (
                        (wqT, cosq_t, sinq_t, qs),
                        (wkT, cosk_t, sink_t, ks),
                    ):
                        for n in range(H):
                            wcol = wcolp.tile([128, KC, 128], PROJ_DT, tag="wcol")
                            nc.sync.dma_start(
                                out=wcol[:],
                                in_=wT[:, n * 128:(n + 1) * 128].rearrange(
                                    "(kc p) n -> p kc n", p=128
                                ),
                            )
                            for th in range(TH):
                                acc = psqk.tile([128, 512], F32, tag="psqk")
                                for k in range(KC):
                                    nc.tensor.matmul(
                                        acc[:],
                                        wcol[:, k, :],
                                        xt[k][:, th * 512:(th + 1) * 512],
                                        start=(k == 0),
                                        stop=(k == KC - 1),
                                    )
                                qt_sb = stagep.tile([128, 512], PROJ_DT, tag="qt_sb")
                                nc.scalar.copy(out=qt_sb[:], in_=acc[:])
                                rot = psrot.tile([128, 512], F32, tag="rot")
                                nc.tensor.matmul(
                                    rot[:], st_t[:], qt_sb[:], start=True, stop=True
                                )
                                t1 = ropep.tile([128, 512], F32, tag="t1")
                                nc.vector.tensor_mul(t1[:], qt_sb[:], cos_t[:])
                                t2 = ropep.tile([128, 512], F32, tag="t2")
                                nc.vector.tensor_mul(t2[:], rot[:], sin_t[:])
                                qr = stagep.tile([128, 512], F32, tag="qr")
                                nc.vector.tensor_add(qr[:], t1[:], t2[:])
                                nc.sync.dma_start(
                                    out=outd[n * 128:(n + 1) * 128,
                                             th * 512:(th + 1) * 512],
                                    in_=qr[:],
                                )

                # --- 1b: V ---
                with (
                    tc.tile_pool(name="wvset", bufs=2) as wvsp,
                    tc.tile_pool(name="vstage", bufs=4) as vstagep,
                    tc.tile_pool(name="psv", bufs=4, space="PSUM") as psv,
                ):
                    for m in range(4):
                        wvs = wvsp.tile([128, KC, 512], PROJ_DT, tag="wvs")
                        nc.sync.dma_start(
                            out=wvs[:],
                            in_=wvT[:, m * 512:(m + 1) * 512].rearrange(
                                "(kc p) n -> p kc n", p=128
                            ),
                        )
                        for tt in range(BPC):
                            acc = psv.tile([128, 512], F32, tag="psv")
                            for k in range(KC):
                                nc.tensor.matmul(
                                    acc[:],
                                    xt[k][:, tt * 128:(tt + 1) * 128],
                                    wvs[:, k, :],
                                    start=(k == 0),
                                    stop=(k == KC - 1),
                                )
                            v_sb = vstagep.tile([128, 512], F32, tag="v_sb")
                            nc.scalar.copy(out=v_sb[:], in_=acc[:])
                            nc.sync.dma_start(
                                out=vs[tt * 128:(tt + 1) * 128,
                                       m * 512:(m + 1) * 512],
                                in_=v_sb[:],
                            )

            # ---------------- phase 2+3: attention + output proj ----------------
            with (
                tc.tile_pool(name="wo", bufs=1) as wop,
                tc.tile_pool(name="qkv", bufs=4) as qkvp,
                tc.tile_pool(name="soft", bufs=3) as softp,
                tc.tile_pool(name="small", bufs=4) as smallp,
                tc.tile_pool(name="attnt", bufs=2) as attntp,
                tc.tile_pool(name="ystage", bufs=3) as ystagep,
                tc.tile_pool(name="psatt", bufs=2, space="PSUM") as psatt,
                tc.tile_pool(name="psy", bufs=2, space="PSUM") as psy,
            ):
                wo_t = wop.tile([128, KC, C], PROJ_DT)
                nc.sync.dma_start(
                    out=wo_t[:],
                    in_=woT.rearrange("(kc p) n -> p kc n", p=128),
                )
                for b in range(BPC):
                    attnT = attntp.tile([128, H, 128], PROJ_DT, tag="attnT")
                    for h in range(H):
                        q_t = qkvp.tile([128, 128], F32, tag="q")
                        nc.sync.dma_start(
                            out=q_t[:],
                            in_=qs[h * 128:(h + 1) * 128, b * 128:(b + 1) * 128],
                        )
                        k_t = qkvp.tile([128, 128], F32, tag="k")
                        nc.sync.dma_start(
                            out=k_t[:],
                            in_=ks[h * 128:(h + 1) * 128, b * 128:(b + 1) * 128],
                        )
                        v_t = qkvp.tile([128, 128], F32, tag="v")
                        nc.sync.dma_start(
                            out=v_t[:],
                            in_=vs[b * 128:(b + 1) * 128, h * 128:(h + 1) * 128],
                        )
                        sc = psatt.tile([128, 128], F32, tag="sc")
                        nc.tensor.matmul(sc[:], q_t[:], k_t[:], start=True, stop=True)
                        masked = softp.tile([128, 128], F32, tag="masked")
                        nc.vector.tensor_add(masked[:], sc[:], mask_t[:])
                        negmax = smallp.tile([128, 1], F32, tag="negmax")
                        nc.vector.reduce_max(
                            out=negmax[:], in_=masked[:],
                            axis=mybir.AxisListType.X, negate=True,
                        )
                        e_t = softp.tile([128, 128], F32, tag="e")
                        sums = smallp.tile([128, 1], F32, tag="sums")
                        nc.scalar.activation(
                            out=e_t[:], in_=masked[:],
                            func=mybir.ActivationFunctionType.Exp,
                            bias=negmax[:], scale=1.0, accum_out=sums[:],
                        )
                        inv = smallp.tile([128, 1], F32, tag="inv")
                        nc.vector.reciprocal(out=inv[:], in_=sums[:])
                        probs = softp.tile([128, 128], F32, tag="probs")
                        nc.vector.tensor_scalar_mul(probs[:], e_t[:], inv[:])
                        pT = psatt.tile([128, 128], F32, tag="pT")
                        nc.tensor.transpose(pT[:], probs[:], id_t[:])
                        pT_sb = softp.tile([128, 128], F32, tag="pT_sb")
                        nc.scalar.copy(out=pT_sb[:], in_=pT[:])
                        pv = psatt.tile([128, 128], F32, tag="pv")
                        nc.tensor.matmul(pv[:], v_t[:], pT_sb[:], start=True, stop=True)
                        nc.scalar.copy(out=attnT[:, h, :], in_=pv[:])
                    for m in range(4):
                        yacc = psy.tile([128, 512], F32, tag="yacc")
                        for h in range(H):
                            nc.tensor.matmul(
                                yacc[:],
                                attnT[:, h, :],
                                wo_t[:, h, m * 512:(m + 1) * 512],
                                start=(h == 0),
                                stop=(h == H - 1),
                            )
                        y_sb = ystagep.tile([128, 512], F32, tag="y_sb")
                        nc.vector.tensor_copy(y_sb[:], yacc[:])
                        nc.sync.dma_start(
                            out=y[b * 128:(b + 1) * 128, m * 512:(m + 1) * 512],
                            in_=y_sb[:],
                        )

    nc.compile()
    return nc


def _prep_inputs(x, freqs_cos, freqs_sin, wq, wk, wv, wo):
    x = np.asarray(x, dtype=np.float32)
    fc = np.asarray(freqs_cos, dtype=np.float32)
    fs = np.asarray(freqs_sin, dtype=np.float32)

    shared = {
        "wqT": np.ascontiguousarray(np.asarray(wq, np.float32).T),
        "wkT": np.ascontiguousarray(np.asarray(wk, np.float32).T),
        "wvT": np.ascontiguousarray(np.asarray(wv, np.float32).T),
        "woT": np.ascontiguousarray(np.asarray(wo, np.float32).T),
    }
    st = np.zeros((128, 128), np.float32)
    for j in range(64):
        st[2 * j + 1, 2 * j] = -1.0
        st[2 * j, 2 * j + 1] = 1.0
    shared["stmat"] = st
    shared["ident"] = np.eye(128, dtype=np.float32)
    shared["maskd"] = np.triu(np.full((128, 128), -1e30, np.float32), k=1)

    cosd = np.repeat(fc.T, 2, axis=0)  # [128, 128]: row d -> cos[t, d//2]
    sind = np.repeat(fs.T, 2, axis=0)
    cos4 = np.ascontiguousarray(np.tile(cosd, (1, 4)))  # [128, 512]
    sin4 = np.ascontiguousarray(np.tile(sind, (1, 4)))
    scale = np.float32(1.0 / np.sqrt(HD))
    shared["cosq"] = cos4 * scale
    shared["sinq"] = sin4 * scale
    shared["cosk"] = cos4
    shared["sink"] = sin4

    in_maps = []
    for i in range(N_CORES):
        shard = x[i * BPC:(i + 1) * BPC].reshape(TOK, C)
        m = dict(shared)
        m["xT"] = np.ascontiguousarray(shard.T)
        in_maps.append(m)
    return in_maps


def _run(inputs, trace=False):
    if "nc" not in _CACHE:
        _CACHE["nc"] = _build()
    nc = _CACHE["nc"]
    in_maps = _prep_inputs(**inputs)
    res = run_bass_kernel_spmd(
        nc, in_maps, core_ids=list(range(N_CORES)), trace=trace
    )
    out = np.empty((B, T, C), np.float32)
    for i in range(N_CORES):
        out[i * BPC:(i + 1) * BPC] = np.asarray(res.results[i]["y"]).reshape(
            BPC, T, C
        )
    return out, res


def kernel(**inputs):
    out, _ = _run(inputs, trace=False)
    return out
